# revision 43
# baseline (speedup 1.0000x reference)
"""TreeLSTM cell (binary children) on 8 Trainium2 NeuronCores.

Data-parallel over the node batch B=262144: each core processes 32768 rows.
All wire tensors are bfloat16 (halves HBM traffic vs f32; the harness
rel-err budget is 2e-2, bf16 noise is ~1e-3).

Host-side data prep (not part of device time):
  - inputs/l_h/r_h are transposed to [128, rows] (contraction dim on
    partitions) and packed into one [128, 3*B_CORE] bf16 tensor.
  - l_c/r_c (and the h/c outputs) are row-permuted so each SBUF
    partition's share of a 2048-row chunk is one contiguous 4KB run in
    HBM, packed into a single [2*B_CORE, 128] bf16 tensor.
  - Weights are re-packed for an all-sigmoid PSUM layout. Per 128-row
    block the A tile holds [i | o | u' | f_l | f_r] (640 cols) where the
    u columns of the weights are pre-scaled by 2 so that
    tanh(x) = 2*sigmoid(2x) - 1 turns the whole tile into one Sigmoid
    activation call. child_h_sum = l_h + r_h is computed on DVE once per
    chunk so the iou matmul runs once on the sum instead of per child.

Per 128-row block r (rows on partitions, gate features on the free axis):
  A[0:640]   = x @ [Wxi Wxo 2Wxu Wfx Wfx]      (start: first per bank)
  A[0:384]  += hs @ [Whi Who 2Whu]
  A[384:512]+= lh @ Whf        A[512:640] += rh @ Whf
  (each matmul is split at PSUM 512-f32 bank boundaries -- a matmul
  output may not cross a bank)
  G = sigmoid(A)   -- ONE ACT call per 3-block super group
Elementwise phase per half-chunk (strided views over G, so the tanh of
half 1 interleaves with the sigmoids of half 2):
  u = 2*u' - 1 (DVE 4x), t1 = i*u (DVE), t2 = fl*lc (DVE),
  t3 = fr*rc (POOL), c = t1+t2+t3 (DVE), tc = tanh(c) (ACT),
  h = o*tc (DVE); output DMA half 1 on the SP queue, half 2 on the ACT
  queue so neither sequencer stalls long on the h-ready semaphore.
Note matmul start=True clears has_written bits for its whole PSUM bank, so
the bank-clearing matmuls (the x pieces of blocks 0 and 2) are emitted
first.  Engine busy (cost model, per core): ACT 187us (binding, ~94% of
its 1 elem/cycle/lane floor for 6 transcendentals/element), DMA 164us
(real HW: ~306 GB/s effective -> ~192us), PE 141us, DVE 124us, POOL 68us.
Measured per-iteration (M1/M2 For_i loop delta, trimmed): ~240us vs the
f32 predecessor's ~335us; rel err 4.7e-3 against the f32 reference.
"""

import numpy as np

import concourse.bass as bass
import concourse.tile as tile
from concourse import mybir

FP = mybir.dt.float32
BF = mybir.dt.bfloat16
N_CORES = 8
B = 262144
D = 128
B_CORE = B // N_CORES          # 32768
CHUNK = 2048                   # rows per DMA chunk
N_CHUNKS = B_CORE // CHUNK     # 16
BLOCKS = CHUNK // 128          # 16 blocks per chunk
SUPERS = [(0, 3), (3, 3), (6, 3), (9, 3), (12, 3), (15, 1)]  # (start, len)
GW = 640                       # gate cols per block: [i o u' fl fr]

Sig = mybir.ActivationFunctionType.Sigmoid
Tanh = mybir.ActivationFunctionType.Tanh

LAST_RESULT = None
_PROGRAM_CACHE = {}


def _split_multi_waits(nc):
    """This walrus codegen allows only one semaphore wait per instruction;
    Tile's scheduler freely attaches several. Hoist the extras onto fresh
    same-engine NoOps placed immediately before the instruction."""
    for blk in nc.main_func.blocks:
        new_insts = []
        for inst in blk.instructions:
            si = inst.sync_info
            if si is not None and si.on_wait and len(si.on_wait) > 1:
                waits = list(si.on_wait)
                upd = list(si.on_update) if si.on_update else []
                for w in waits[:-1]:
                    nop = mybir.InstNoOp(
                        name=nc.get_next_instruction_name(), ins=[], outs=[])
                    nop.engine = inst.engine
                    nop.sync_info = mybir.SyncInfo(on_wait=[w], on_update=[])
                    nc.register_instruction(nop)
                    new_insts.append(nop)
                inst.sync_info = mybir.SyncInfo(
                    on_wait=[waits[-1]], on_update=upd)
            new_insts.append(inst)
        blk.instructions[:] = new_insts


def _build_program(with_bias: bool, bench_loops: int = 0):
    nc = bass.Bass()
    # xT/lhT/rhT chunk-major: [chunk, partition, seg, row] so each
    # partition's share of a chunk DMA is one contiguous 12KB run
    tpk = nc.dram_tensor("tpk", [N_CHUNKS * D * 3, CHUNK], BF,
                         kind="ExternalInput")
    # perm(l_c)/perm(r_c) chunk-major: one contiguous 8KB run/partition
    npk = nc.dram_tensor("npk", [2 * B_CORE, D], BF, kind="ExternalInput")
    WxA = nc.dram_tensor("WxA", [D, GW], BF, kind="ExternalInput")
    WhI = nc.dram_tensor("WhI", [D, 384], BF, kind="ExternalInput")
    Whf = nc.dram_tensor("Whf", [D, 128], BF, kind="ExternalInput")
    if with_bias:
        biasA = nc.dram_tensor("biasA", [1, GW], BF, kind="ExternalInput")
        ones = nc.dram_tensor("ones", [1, 128], BF, kind="ExternalInput")
    # [perm(h); perm(c)] packed along rows
    hc = nc.dram_tensor("hc", [2 * B_CORE, D], BF, kind="ExternalOutput")

    # chunk views: per partition ONE contiguous run per chunk DMA
    tpk_v = tpk[:].rearrange("(c p s) n -> c p s n",
                             c=N_CHUNKS, p=D)                    # [NCH,128,3,2048]
    npk_v = npk[:].rearrange("(c p s j) f -> c p s (j f)",
                             p=D, s=2, j=BLOCKS)                 # [NCH,128,2,2048]
    hc_v = hc[:].rearrange("(c p s j) f -> c p s (j f)",
                           p=D, s=2, j=BLOCKS)

    with tile.TileContext(nc) as tc:
        with (
            tc.tile_pool(name="w", bufs=1) as wpool,
            tc.tile_pool(name="ins", bufs=2) as inpool,
            tc.tile_pool(name="hs", bufs=2) as hspool,
            tc.tile_pool(name="tc", bufs=2) as tcpool,
            tc.tile_pool(name="outs", bufs=2) as outpool,
            tc.tile_pool(name="gates", bufs=2) as gpool,
            tc.tile_pool(name="temps", bufs=2) as tpool,
            tc.tile_pool(name="psA", bufs=2, space=bass.MemorySpace.PSUM) as apool,
        ):
            w_x = wpool.tile([D, GW], BF)
            nc.sync.dma_start(w_x[:], WxA[:])
            w_hi = wpool.tile([D, 384], BF)
            nc.sync.dma_start(w_hi[:], WhI[:])
            w_hf = wpool.tile([D, 128], BF)
            nc.sync.dma_start(w_hf[:], Whf[:])
            bA = one_t = None
            if with_bias:
                bA = wpool.tile([1, GW], BF)
                nc.sync.dma_start(bA[:], biasA[:])
                one_t = wpool.tile([1, 128], BF)
                nc.sync.dma_start(one_t[:], ones[:])

            def emit_chunk(ch):
                tp = inpool.tile([D, 3, CHUNK], BF, tag="tp")
                nc.sync.dma_start(tp[:], tpk_v[ch])
                np_t = inpool.tile([D, 2, CHUNK], BF, tag="np")
                nc.sync.dma_start(np_t[:], npk_v[ch])
                out_t = outpool.tile([D, 2, CHUNK], BF, tag="out")
                ht = out_t[:, 0, :]
                ct = out_t[:, 1, :]
                lct = np_t[:, 0, :]
                rct = np_t[:, 1, :]

                # child_h_sum for the whole chunk in one DVE op
                hs_t = hspool.tile([D, CHUNK], BF, tag="hs")
                nc.vector.tensor_add(hs_t[:], tp[:, 1, :], tp[:, 2, :])

                # gate tile for the whole chunk: [i o u' fl fr] per block
                G = gpool.tile([D, BLOCKS, GW], BF, tag="G")

                for j0, ns in SUPERS:
                    A = apool.tile([D, 3 * GW], FP, tag="A")

                    def mm(k, stat, w, w0, w1, wbase=0, start=False,
                           stop=False):
                        """Matmul into tile cols [k*GW+w0, k*GW+w1) from
                        weight cols starting at wbase, split at PSUM bank
                        boundaries (matmul output must stay within one
                        512-f32 bank)."""
                        c0, c1 = k * GW + w0, k * GW + w1
                        while c0 < c1:
                            ce = min(c1, (c0 // 512 + 1) * 512)
                            ww0 = wbase + (c0 - (k * GW + w0))
                            nc.tensor.matmul(
                                A[:, c0:ce], stat, w[:, ww0:ww0 + ce - c0],
                                start=start, stop=stop,
                                skip_group_check=True)
                            c0 = ce

                    # bank-clearing writers first: the x (or bias) pieces of
                    # blocks 0 and 2 jointly touch every bank of the tile
                    order = [0, 2, 1][:ns] if ns >= 3 else list(range(ns))
                    if with_bias:
                        for idx, k in enumerate(order):
                            mm(k, one_t[:], bA, 0, GW, start=(idx < 2))
                        for k in order:
                            xb = tp[:, 0, (j0 + k) * 128:(j0 + k + 1) * 128]
                            mm(k, xb, w_x, 0, GW)
                    else:
                        for idx, k in enumerate(order):
                            xb = tp[:, 0, (j0 + k) * 128:(j0 + k + 1) * 128]
                            mm(k, xb, w_x, 0, GW,
                               start=(idx < (2 if ns >= 3 else ns)))
                    for k in range(ns):
                        hb = hs_t[:, (j0 + k) * 128:(j0 + k + 1) * 128]
                        mm(k, hb, w_hi, 0, 384)
                    for k in range(ns):
                        lb = tp[:, 1, (j0 + k) * 128:(j0 + k + 1) * 128]
                        mm(k, lb, w_hf, 384, 512, wbase=0)
                    for k in range(ns):
                        rb = tp[:, 2, (j0 + k) * 128:(j0 + k + 1) * 128]
                        mm(k, rb, w_hf, 512, GW, wbase=0, stop=True)

                    # ONE sigmoid over the whole super tile (PSUM -> SBUF)
                    nc.scalar.activation(
                        G[:, j0:j0 + ns, :].rearrange("p k w -> p (k w)"),
                        A[:, 0:ns * GW], Sig)

                # elementwise phase at half-chunk granularity so the tanh
                # of half 1 interleaves with the sigmoids of half 2 and the
                # chunk tail stays short
                T = tpool.tile([D, 4, CHUNK], BF, tag="T")
                Tv = T[:].rearrange("p r (k w) -> p r k w", w=128)
                # dedicated tanh scratch: aliasing it onto hs_t couples the
                # chunk tail (h-mul) to the hs-add -> matmuls of chunk n+2
                # through the 2-deep hs pool
                tc_t = tcpool.tile([D, CHUNK], BF, tag="tc")
                tcx = tc_t[:].rearrange("p (k w) -> p k w", w=128)

                lcs = lct.rearrange("p (k w) -> p k w", w=128)
                rcs = rct.rearrange("p (k w) -> p k w", w=128)
                cts = ct.rearrange("p (k w) -> p k w", w=128)
                hts = ht.rearrange("p (k w) -> p k w", w=128)

                for b0, b1 in ((0, 9), (9, 16)):
                    ks = slice(b0, b1)
                    i_ = G[:, ks, 0:128]
                    o_ = G[:, ks, 128:256]
                    up = G[:, ks, 256:384]
                    fl = G[:, ks, 384:512]
                    fr = G[:, ks, 512:640]
                    u_ = Tv[:, 0, ks]
                    t1 = Tv[:, 1, ks]
                    t2 = Tv[:, 2, ks]
                    t3 = Tv[:, 3, ks]

                    nc.vector.tensor_scalar(u_, up, 2.0, -1.0,
                                            mybir.AluOpType.mult,
                                            mybir.AluOpType.add)
                    nc.vector.tensor_mul(t1, i_, u_)
                    nc.vector.tensor_mul(t2, fl, lcs[:, ks])
                    nc.gpsimd.tensor_mul(t3, fr, rcs[:, ks])
                    nc.vector.tensor_add(t1, t1, t2)
                    nc.vector.tensor_add(cts[:, ks], t1, t3)
                    nc.scalar.activation(tcx[:, ks], cts[:, ks], Tanh)
                    nc.vector.tensor_mul(hts[:, ks], o_, tcx[:, ks])

                    rows = slice(b0 * 128, b1 * 128)
                    # half 1 on the SP queue, half 2 on the ACT queue: the
                    # late h-ready wait never heads the queue that is about
                    # to dispatch the next chunk's sigmoids (ACT) or input
                    # prefetches (SP) at the wrong moment
                    eng = nc.sync if b0 == 0 else nc.scalar
                    eng.dma_start(hc_v[ch][:, :, rows],
                                  out_t[:, :, rows])

            if bench_loops:
                with tc.For_i(0, bench_loops, 1):
                    for ch in range(N_CHUNKS):
                        emit_chunk(ch)
            else:
                for ch in range(N_CHUNKS):
                    emit_chunk(ch)

    _split_multi_waits(nc)
    return nc


def _get_program(with_bias: bool):
    if with_bias not in _PROGRAM_CACHE:
        _PROGRAM_CACHE[with_bias] = _build_program(with_bias)
    return _PROGRAM_CACHE[with_bias]


class _Runner:
    """Compiled 8-core SPMD executable for one Bass program (the jit body
    mirrors concourse.bass2jax.run_bass_via_pjrt, but is built once and
    reused so repeat kernel() calls and benchmarking skip recompilation)."""

    def __init__(self, nc):
        import jax
        from jax.sharding import Mesh, PartitionSpec, NamedSharding
        from jax.experimental.shard_map import shard_map
        from concourse import bass2jax

        bass2jax.install_neuronx_cc_hook()
        self.jax = jax
        part_name = nc.partition_id_tensor.name if nc.partition_id_tensor else None
        in_names, out_names, out_avals, zero_outs = [], [], [], []
        for alloc in nc.m.functions[0].allocations:
            if not isinstance(alloc, mybir.MemoryLocationSet):
                continue
            name = alloc.memorylocations[0].name
            if alloc.kind == "ExternalInput":
                if name != part_name:
                    in_names.append(name)
            elif alloc.kind == "ExternalOutput":
                out_names.append(name)
                shape = tuple(alloc.tensor_shape)
                dtype = mybir.dt.np(alloc.dtype)
                out_avals.append(jax.core.ShapedArray(shape, dtype))
                zero_outs.append(np.zeros(shape, dtype))
        self.in_names = list(in_names)
        self.out_names = out_names
        self.out_avals = out_avals
        self.zero_outs = zero_outs
        n_params = len(in_names)
        all_in_names = in_names + out_names
        if part_name is not None:
            all_in_names = all_in_names + [part_name]

        def _body(*args):
            operands = list(args)
            if part_name is not None:
                operands.append(bass2jax.partition_id_tensor())
            outs = bass2jax._bass_exec_p.bind(
                *operands,
                out_avals=tuple(out_avals),
                in_names=tuple(all_in_names),
                out_names=tuple(out_names),
                lowering_input_output_aliases=(),
                sim_require_finite=True,
                sim_require_nnan=True,
                nc=nc,
            )
            return tuple(outs)

        devices = jax.devices()[:N_CORES]
        self.mesh = Mesh(np.asarray(devices), ("core",))
        self.sharding = NamedSharding(self.mesh, PartitionSpec("core"))
        in_specs = (PartitionSpec("core"),) * (n_params + len(out_names))
        out_specs = (PartitionSpec("core"),) * len(out_names)
        self.fn = jax.jit(
            shard_map(_body, mesh=self.mesh, in_specs=in_specs,
                      out_specs=out_specs, check_rep=False),
            keep_unused=True,
        )

    def stage(self, in_maps):
        """device_put concatenated inputs (+ zero output buffers) once."""
        jax = self.jax
        concat = [
            np.concatenate([m[name] for m in in_maps], axis=0)
            for name in self.in_names
        ]
        concat += [
            np.zeros((N_CORES * z.shape[0], *z.shape[1:]), z.dtype)
            for z in self.zero_outs
        ]
        return [jax.device_put(a, self.sharding) for a in concat]

    def run(self, staged):
        outs = self.fn(*staged)
        self.jax.block_until_ready(outs)
        return outs

    def results(self, outs):
        per_core = []
        for c in range(N_CORES):
            d = {}
            for i, name in enumerate(self.out_names):
                d[name] = np.asarray(outs[i]).reshape(
                    N_CORES, *self.out_avals[i].shape)[c]
            per_core.append(d)
        return per_core


def _get_runner(with_bias: bool):
    key = ("runner", with_bias)
    if key not in _PROGRAM_CACHE:
        _PROGRAM_CACHE[key] = _Runner(_get_program(with_bias))
    return _PROGRAM_CACHE[key]


def _bf16(a):
    return np.asarray(a).astype(mybir.dt.np(BF))


def _pack_tp(x, lh, rh):
    """[c p s n] chunk-major packing of the transposed x/l_h/r_h."""
    a = np.stack([x.T, lh.T, rh.T], axis=1)          # [128, 3, B_CORE]
    a = a.reshape(D, 3, N_CHUNKS, CHUNK).transpose(2, 0, 1, 3)
    return np.ascontiguousarray(a.reshape(N_CHUNKS * D * 3, CHUNK))


def _pack_np(lc, rc):
    """[c p s j f] chunk-major packing of the row-permuted l_c/r_c
    (row c*2048 + j*128 + p of the original sits at [c, p, s, j])."""
    l4 = lc.reshape(N_CHUNKS, BLOCKS, D, D).swapaxes(1, 2)   # [c p j f]
    r4 = rc.reshape(N_CHUNKS, BLOCKS, D, D).swapaxes(1, 2)
    s = np.stack([l4, r4], axis=2)                   # [c p s j f]
    return np.ascontiguousarray(s.reshape(2 * B_CORE, D))


def _unpack_hc(a):
    """Inverse of the hc [c p s j f] layout -> (h, c) row-major."""
    a5 = a.reshape(N_CHUNKS, D, 2, BLOCKS, D)
    h = a5[:, :, 0].swapaxes(1, 2).reshape(B_CORE, D)
    c = a5[:, :, 1].swapaxes(1, 2).reshape(B_CORE, D)
    return h, c


def kernel(l_h, l_c, r_h, r_c, inputs, W_ioux, b_ioux, W_iouh, b_iouh,
           W_fx, b_fx, W_fh, b_fh):
    global LAST_RESULT
    f32 = lambda a: np.ascontiguousarray(np.asarray(a), dtype=np.float32)
    l_h, l_c, r_h, r_c, inputs = map(f32, (l_h, l_c, r_h, r_c, inputs))
    W_ioux, W_iouh, W_fx, W_fh = map(f32, (W_ioux, W_iouh, W_fx, W_fh))
    b_ioux, b_iouh, b_fx, b_fh = map(f32, (b_ioux, b_iouh, b_fx, b_fh))

    with_bias = bool(np.any(b_ioux) or np.any(b_iouh)
                     or np.any(b_fx) or np.any(b_fh))

    # all-sigmoid gate layout [i | o | u' | fl | fr]; u cols scaled by 2
    # so tanh(x) = 2*sigmoid(2x) - 1
    WxA = np.concatenate(
        [W_ioux[:, 0:256], 2.0 * W_ioux[:, 256:384], W_fx, W_fx], axis=1)
    WhI = np.concatenate(
        [W_iouh[:, 0:256], 2.0 * W_iouh[:, 256:384]], axis=1)
    if with_bias:
        b_iou = b_ioux + b_iouh
        b_f = b_fx + b_fh
        biasA = np.concatenate(
            [b_iou[0:256], 2.0 * b_iou[256:384], b_f, b_f]).reshape(1, GW)
        ones = np.ones((1, 128), dtype=np.float32)

    in_maps = []
    for core in range(N_CORES):
        sl = slice(core * B_CORE, (core + 1) * B_CORE)
        m = {
            "tpk": _bf16(_pack_tp(inputs[sl], l_h[sl], r_h[sl])),
            "npk": _bf16(_pack_np(l_c[sl], r_c[sl])),
            "WxA": _bf16(WxA),
            "WhI": _bf16(WhI),
            "Whf": _bf16(W_fh),
        }
        if with_bias:
            m["biasA"] = _bf16(biasA)
            m["ones"] = _bf16(ones)
        in_maps.append(m)

    runner = _get_runner(with_bias)
    staged = runner.stage(in_maps)
    outs = runner.run(staged)
    per_core = runner.results(outs)
    LAST_RESULT = (runner, staged)
    hs, cs = [], []
    for d in per_core:
        h, c = _unpack_hc(d["hc"].astype(np.float32))
        hs.append(h)
        cs.append(c)
    return np.concatenate(hs, axis=0), np.concatenate(cs, axis=0)


# revision 44
# speedup vs baseline: 1.0658x; 1.0658x over previous
"""TreeLSTM cell (binary children) on 8 Trainium2 NeuronCores.

Data-parallel over the node batch B=262144: each core processes 32768 rows.
All wire tensors are bfloat16 (halves HBM traffic vs f32; the harness
rel-err budget is 2e-2, bf16 noise is ~1e-3).

Host-side data prep (not part of device time):
  - inputs/l_h/r_h are transposed to [128, rows] (contraction dim on
    partitions) and packed into one [128, 3*B_CORE] bf16 tensor.
  - l_c/r_c (and the h/c outputs) are row-permuted so each SBUF
    partition's share of a 2048-row chunk is one contiguous 4KB run in
    HBM, packed into a single [2*B_CORE, 128] bf16 tensor.
  - Weights are re-packed for an all-sigmoid PSUM layout. Per 128-row
    block the A tile holds [i | o | u' | f_l | f_r] (640 cols) where the
    u columns of the weights are pre-scaled by 2 so that
    tanh(x) = 2*sigmoid(2x) - 1 turns the whole tile into one Sigmoid
    activation call. child_h_sum = l_h + r_h is computed on DVE once per
    chunk so the iou matmul runs once on the sum instead of per child.

Per 128-row block r (rows on partitions, gate features on the free axis):
  A[0:640]   = x @ [Wxi Wxo 2Wxu Wfx Wfx]      (start: first per bank)
  A[0:384]  += hs @ [Whi Who 2Whu]
  A[384:512]+= lh @ Whf        A[512:640] += rh @ Whf
  (each matmul is split at PSUM 512-f32 bank boundaries -- a matmul
  output may not cross a bank)
  G = sigmoid(A)   -- ONE ACT call per 3-block super group
Elementwise phase per half-chunk (strided views over G, so the tanh of
half 1 interleaves with the sigmoids of half 2):
  u = 2*u' - 1 (DVE 4x), t1 = i*u (DVE), t2 = fl*lc (DVE),
  t3 = fr*rc (POOL), c = t1+t2+t3 (DVE), tc = tanh(c) (ACT),
  h = o*tc (DVE); output DMA half 1 on the SP queue, half 2 on the ACT
  queue so neither sequencer stalls long on the h-ready semaphore.
Note matmul start=True clears has_written bits for its whole PSUM bank, so
the bank-clearing matmuls (the x pieces of blocks 0 and 2) are emitted
first.  Engine busy (cost model, per core): ACT 187us (binding, ~94% of
its 1 elem/cycle/lane floor for 6 transcendentals/element), DMA 164us
(real HW: ~306 GB/s effective -> ~192us), PE 141us, DVE 124us, POOL 68us.
Measured per-iteration (M1/M2 For_i loop delta, trimmed): ~240us vs the
f32 predecessor's ~335us; rel err 4.7e-3 against the f32 reference.
"""

import numpy as np

import concourse.bass as bass
import concourse.tile as tile
from concourse import mybir

FP = mybir.dt.float32
BF = mybir.dt.bfloat16
N_CORES = 8
B = 262144
D = 128
B_CORE = B // N_CORES          # 32768
CHUNK = 2048                   # rows per DMA chunk
N_CHUNKS = B_CORE // CHUNK     # 16
BLOCKS = CHUNK // 128          # 16 blocks per chunk
SUPERS = [(0, 3), (3, 3), (6, 3), (9, 3), (12, 3), (15, 1)]  # (start, len)
GW = 640                       # gate cols per block: [i o u' fl fr]

Sig = mybir.ActivationFunctionType.Sigmoid
Tanh = mybir.ActivationFunctionType.Tanh

LAST_RESULT = None
_PROGRAM_CACHE = {}


def _split_multi_waits(nc):
    """This walrus codegen allows only one semaphore wait per instruction;
    Tile's scheduler freely attaches several. Hoist the extras onto fresh
    same-engine NoOps placed immediately before the instruction."""
    for blk in nc.main_func.blocks:
        new_insts = []
        for inst in blk.instructions:
            si = inst.sync_info
            if si is not None and si.on_wait and len(si.on_wait) > 1:
                waits = list(si.on_wait)
                upd = list(si.on_update) if si.on_update else []
                for w in waits[:-1]:
                    nop = mybir.InstNoOp(
                        name=nc.get_next_instruction_name(), ins=[], outs=[])
                    nop.engine = inst.engine
                    nop.sync_info = mybir.SyncInfo(on_wait=[w], on_update=[])
                    nc.register_instruction(nop)
                    new_insts.append(nop)
                inst.sync_info = mybir.SyncInfo(
                    on_wait=[waits[-1]], on_update=upd)
            new_insts.append(inst)
        blk.instructions[:] = new_insts


def _build_program(with_bias: bool, bench_loops: int = 0):
    nc = bass.Bass()
    # xT/lhT/rhT chunk-major: [chunk, partition, seg, row] so each
    # partition's share of a chunk DMA is one contiguous 12KB run
    tpk = nc.dram_tensor("tpk", [N_CHUNKS * D * 3, CHUNK], BF,
                         kind="ExternalInput")
    # perm(l_c)/perm(r_c) chunk-major: one contiguous 8KB run/partition
    npk = nc.dram_tensor("npk", [2 * B_CORE, D], BF, kind="ExternalInput")
    WxA = nc.dram_tensor("WxA", [D, GW], BF, kind="ExternalInput")
    WhI = nc.dram_tensor("WhI", [D, 384], BF, kind="ExternalInput")
    Whf = nc.dram_tensor("Whf", [D, 128], BF, kind="ExternalInput")
    if with_bias:
        biasA = nc.dram_tensor("biasA", [1, GW], BF, kind="ExternalInput")
        ones = nc.dram_tensor("ones", [1, 128], BF, kind="ExternalInput")
    # [perm(h); perm(c)] packed along rows
    hc = nc.dram_tensor("hc", [2 * B_CORE, D], BF, kind="ExternalOutput")

    # chunk views: per partition ONE contiguous run per chunk DMA
    tpk_v = tpk[:].rearrange("(c p s) n -> c p s n",
                             c=N_CHUNKS, p=D)                    # [NCH,128,3,2048]
    npk_v = npk[:].rearrange("(c p s j) f -> c p s (j f)",
                             p=D, s=2, j=BLOCKS)                 # [NCH,128,2,2048]
    hc_v = hc[:].rearrange("(c p s j) f -> c p s (j f)",
                           p=D, s=2, j=BLOCKS)

    with tile.TileContext(nc) as tc:
        with (
            tc.tile_pool(name="w", bufs=1) as wpool,
            tc.tile_pool(name="ins", bufs=2) as inpool,
            tc.tile_pool(name="hs", bufs=2) as hspool,
            tc.tile_pool(name="outs", bufs=2) as outpool,
            tc.tile_pool(name="gates", bufs=2) as gpool,
            tc.tile_pool(name="temps", bufs=2) as tpool,
            tc.tile_pool(name="psA", bufs=2, space=bass.MemorySpace.PSUM) as apool,
        ):
            w_x = wpool.tile([D, GW], BF)
            nc.sync.dma_start(w_x[:], WxA[:])
            w_hi = wpool.tile([D, 384], BF)
            nc.sync.dma_start(w_hi[:], WhI[:])
            w_hf = wpool.tile([D, 128], BF)
            nc.sync.dma_start(w_hf[:], Whf[:])
            bA = one_t = None
            if with_bias:
                bA = wpool.tile([1, GW], BF)
                nc.sync.dma_start(bA[:], biasA[:])
                one_t = wpool.tile([1, 128], BF)
                nc.sync.dma_start(one_t[:], ones[:])

            def emit_chunk(ch):
                tp = inpool.tile([D, 3, CHUNK], BF, tag="tp")
                nc.sync.dma_start(tp[:], tpk_v[ch])
                np_t = inpool.tile([D, 2, CHUNK], BF, tag="np")
                nc.sync.dma_start(np_t[:], npk_v[ch])
                out_t = outpool.tile([D, 2, CHUNK], BF, tag="out")
                ht = out_t[:, 0, :]
                ct = out_t[:, 1, :]
                lct = np_t[:, 0, :]
                rct = np_t[:, 1, :]

                # child_h_sum for the whole chunk in one DVE op
                hs_t = hspool.tile([D, CHUNK], BF, tag="hs")
                nc.vector.tensor_add(hs_t[:], tp[:, 1, :], tp[:, 2, :])

                # gate tile for the whole chunk: [i o u' fl fr] per block
                G = gpool.tile([D, BLOCKS, GW], BF, tag="G")

                for j0, ns in SUPERS:
                    A = apool.tile([D, 3 * GW], FP, tag="A")

                    def mm(k, stat, w, w0, w1, wbase=0, start=False,
                           stop=False):
                        """Matmul into tile cols [k*GW+w0, k*GW+w1) from
                        weight cols starting at wbase, split at PSUM bank
                        boundaries (matmul output must stay within one
                        512-f32 bank)."""
                        c0, c1 = k * GW + w0, k * GW + w1
                        while c0 < c1:
                            ce = min(c1, (c0 // 512 + 1) * 512)
                            ww0 = wbase + (c0 - (k * GW + w0))
                            nc.tensor.matmul(
                                A[:, c0:ce], stat, w[:, ww0:ww0 + ce - c0],
                                start=start, stop=stop,
                                skip_group_check=True)
                            c0 = ce

                    # bank-clearing writers first: the x (or bias) pieces of
                    # blocks 0 and 2 jointly touch every bank of the tile
                    order = [0, 2, 1][:ns] if ns >= 3 else list(range(ns))
                    if with_bias:
                        for idx, k in enumerate(order):
                            mm(k, one_t[:], bA, 0, GW, start=(idx < 2))
                        for k in order:
                            xb = tp[:, 0, (j0 + k) * 128:(j0 + k + 1) * 128]
                            mm(k, xb, w_x, 0, GW)
                    else:
                        for idx, k in enumerate(order):
                            xb = tp[:, 0, (j0 + k) * 128:(j0 + k + 1) * 128]
                            mm(k, xb, w_x, 0, GW,
                               start=(idx < (2 if ns >= 3 else ns)))
                    for k in range(ns):
                        hb = hs_t[:, (j0 + k) * 128:(j0 + k + 1) * 128]
                        mm(k, hb, w_hi, 0, 384)
                    for k in range(ns):
                        lb = tp[:, 1, (j0 + k) * 128:(j0 + k + 1) * 128]
                        mm(k, lb, w_hf, 384, 512, wbase=0)
                    for k in range(ns):
                        rb = tp[:, 2, (j0 + k) * 128:(j0 + k + 1) * 128]
                        mm(k, rb, w_hf, 512, GW, wbase=0, stop=True)

                    # ONE sigmoid over the whole super tile (PSUM -> SBUF)
                    nc.scalar.activation(
                        G[:, j0:j0 + ns, :].rearrange("p k w -> p (k w)"),
                        A[:, 0:ns * GW], Sig)

                # elementwise phase at half-chunk granularity so the tanh
                # of half 1 interleaves with the sigmoids of half 2 and the
                # chunk tail stays short
                T = tpool.tile([D, 4, CHUNK], BF, tag="T")
                Tv = T[:].rearrange("p r (k w) -> p r k w", w=128)
                tcx = hs_t[:].rearrange("p (k w) -> p k w", w=128)  # reuse

                lcs = lct.rearrange("p (k w) -> p k w", w=128)
                rcs = rct.rearrange("p (k w) -> p k w", w=128)
                cts = ct.rearrange("p (k w) -> p k w", w=128)
                hts = ht.rearrange("p (k w) -> p k w", w=128)

                for b0, b1 in ((0, 9), (9, 16)):
                    ks = slice(b0, b1)
                    i_ = G[:, ks, 0:128]
                    o_ = G[:, ks, 128:256]
                    up = G[:, ks, 256:384]
                    fl = G[:, ks, 384:512]
                    fr = G[:, ks, 512:640]
                    u_ = Tv[:, 0, ks]
                    t1 = Tv[:, 1, ks]
                    t2 = Tv[:, 2, ks]
                    t3 = Tv[:, 3, ks]

                    nc.vector.tensor_scalar(u_, up, 2.0, -1.0,
                                            mybir.AluOpType.mult,
                                            mybir.AluOpType.add)
                    nc.vector.tensor_mul(t1, i_, u_)
                    nc.vector.tensor_mul(t2, fl, lcs[:, ks])
                    nc.gpsimd.tensor_mul(t3, fr, rcs[:, ks])
                    nc.vector.tensor_add(t1, t1, t2)
                    nc.vector.tensor_add(cts[:, ks], t1, t3)
                    nc.scalar.activation(tcx[:, ks], cts[:, ks], Tanh)
                    nc.vector.tensor_mul(hts[:, ks], o_, tcx[:, ks])

                    rows = slice(b0 * 128, b1 * 128)
                    # half 1 on the SP queue, half 2 on the ACT queue: the
                    # late h-ready wait never heads the queue that is about
                    # to dispatch the next chunk's sigmoids (ACT) or input
                    # prefetches (SP) at the wrong moment
                    eng = nc.sync if b0 == 0 else nc.scalar
                    eng.dma_start(hc_v[ch][:, :, rows],
                                  out_t[:, :, rows])

            if bench_loops:
                with tc.For_i(0, bench_loops, 1):
                    for ch in range(N_CHUNKS):
                        emit_chunk(ch)
            else:
                for ch in range(N_CHUNKS):
                    emit_chunk(ch)

    _split_multi_waits(nc)
    return nc


def _get_program(with_bias: bool):
    if with_bias not in _PROGRAM_CACHE:
        _PROGRAM_CACHE[with_bias] = _build_program(with_bias)
    return _PROGRAM_CACHE[with_bias]


class _Runner:
    """Compiled 8-core SPMD executable for one Bass program (the jit body
    mirrors concourse.bass2jax.run_bass_via_pjrt, but is built once and
    reused so repeat kernel() calls and benchmarking skip recompilation)."""

    def __init__(self, nc):
        import jax
        from jax.sharding import Mesh, PartitionSpec, NamedSharding
        from jax.experimental.shard_map import shard_map
        from concourse import bass2jax

        bass2jax.install_neuronx_cc_hook()
        self.jax = jax
        part_name = nc.partition_id_tensor.name if nc.partition_id_tensor else None
        in_names, out_names, out_avals, zero_outs = [], [], [], []
        for alloc in nc.m.functions[0].allocations:
            if not isinstance(alloc, mybir.MemoryLocationSet):
                continue
            name = alloc.memorylocations[0].name
            if alloc.kind == "ExternalInput":
                if name != part_name:
                    in_names.append(name)
            elif alloc.kind == "ExternalOutput":
                out_names.append(name)
                shape = tuple(alloc.tensor_shape)
                dtype = mybir.dt.np(alloc.dtype)
                out_avals.append(jax.core.ShapedArray(shape, dtype))
                zero_outs.append(np.zeros(shape, dtype))
        self.in_names = list(in_names)
        self.out_names = out_names
        self.out_avals = out_avals
        self.zero_outs = zero_outs
        n_params = len(in_names)
        all_in_names = in_names + out_names
        if part_name is not None:
            all_in_names = all_in_names + [part_name]

        def _body(*args):
            operands = list(args)
            if part_name is not None:
                operands.append(bass2jax.partition_id_tensor())
            outs = bass2jax._bass_exec_p.bind(
                *operands,
                out_avals=tuple(out_avals),
                in_names=tuple(all_in_names),
                out_names=tuple(out_names),
                lowering_input_output_aliases=(),
                sim_require_finite=True,
                sim_require_nnan=True,
                nc=nc,
            )
            return tuple(outs)

        devices = jax.devices()[:N_CORES]
        self.mesh = Mesh(np.asarray(devices), ("core",))
        self.sharding = NamedSharding(self.mesh, PartitionSpec("core"))
        in_specs = (PartitionSpec("core"),) * (n_params + len(out_names))
        out_specs = (PartitionSpec("core"),) * len(out_names)
        self.fn = jax.jit(
            shard_map(_body, mesh=self.mesh, in_specs=in_specs,
                      out_specs=out_specs, check_rep=False),
            keep_unused=True,
        )

    def stage(self, in_maps):
        """device_put concatenated inputs (+ zero output buffers) once."""
        jax = self.jax
        concat = [
            np.concatenate([m[name] for m in in_maps], axis=0)
            for name in self.in_names
        ]
        concat += [
            np.zeros((N_CORES * z.shape[0], *z.shape[1:]), z.dtype)
            for z in self.zero_outs
        ]
        return [jax.device_put(a, self.sharding) for a in concat]

    def run(self, staged):
        outs = self.fn(*staged)
        self.jax.block_until_ready(outs)
        return outs

    def results(self, outs):
        per_core = []
        for c in range(N_CORES):
            d = {}
            for i, name in enumerate(self.out_names):
                d[name] = np.asarray(outs[i]).reshape(
                    N_CORES, *self.out_avals[i].shape)[c]
            per_core.append(d)
        return per_core


def _get_runner(with_bias: bool):
    key = ("runner", with_bias)
    if key not in _PROGRAM_CACHE:
        _PROGRAM_CACHE[key] = _Runner(_get_program(with_bias))
    return _PROGRAM_CACHE[key]


def _bf16(a):
    return np.asarray(a).astype(mybir.dt.np(BF))


def _pack_tp(x, lh, rh):
    """[c p s n] chunk-major packing of the transposed x/l_h/r_h."""
    a = np.stack([x.T, lh.T, rh.T], axis=1)          # [128, 3, B_CORE]
    a = a.reshape(D, 3, N_CHUNKS, CHUNK).transpose(2, 0, 1, 3)
    return np.ascontiguousarray(a.reshape(N_CHUNKS * D * 3, CHUNK))


def _pack_np(lc, rc):
    """[c p s j f] chunk-major packing of the row-permuted l_c/r_c
    (row c*2048 + j*128 + p of the original sits at [c, p, s, j])."""
    l4 = lc.reshape(N_CHUNKS, BLOCKS, D, D).swapaxes(1, 2)   # [c p j f]
    r4 = rc.reshape(N_CHUNKS, BLOCKS, D, D).swapaxes(1, 2)
    s = np.stack([l4, r4], axis=2)                   # [c p s j f]
    return np.ascontiguousarray(s.reshape(2 * B_CORE, D))


def _unpack_hc(a):
    """Inverse of the hc [c p s j f] layout -> (h, c) row-major."""
    a5 = a.reshape(N_CHUNKS, D, 2, BLOCKS, D)
    h = a5[:, :, 0].swapaxes(1, 2).reshape(B_CORE, D)
    c = a5[:, :, 1].swapaxes(1, 2).reshape(B_CORE, D)
    return h, c


def kernel(l_h, l_c, r_h, r_c, inputs, W_ioux, b_ioux, W_iouh, b_iouh,
           W_fx, b_fx, W_fh, b_fh):
    global LAST_RESULT
    f32 = lambda a: np.ascontiguousarray(np.asarray(a), dtype=np.float32)
    l_h, l_c, r_h, r_c, inputs = map(f32, (l_h, l_c, r_h, r_c, inputs))
    W_ioux, W_iouh, W_fx, W_fh = map(f32, (W_ioux, W_iouh, W_fx, W_fh))
    b_ioux, b_iouh, b_fx, b_fh = map(f32, (b_ioux, b_iouh, b_fx, b_fh))

    with_bias = bool(np.any(b_ioux) or np.any(b_iouh)
                     or np.any(b_fx) or np.any(b_fh))

    # all-sigmoid gate layout [i | o | u' | fl | fr]; u cols scaled by 2
    # so tanh(x) = 2*sigmoid(2x) - 1
    WxA = np.concatenate(
        [W_ioux[:, 0:256], 2.0 * W_ioux[:, 256:384], W_fx, W_fx], axis=1)
    WhI = np.concatenate(
        [W_iouh[:, 0:256], 2.0 * W_iouh[:, 256:384]], axis=1)
    if with_bias:
        b_iou = b_ioux + b_iouh
        b_f = b_fx + b_fh
        biasA = np.concatenate(
            [b_iou[0:256], 2.0 * b_iou[256:384], b_f, b_f]).reshape(1, GW)
        ones = np.ones((1, 128), dtype=np.float32)

    in_maps = []
    for core in range(N_CORES):
        sl = slice(core * B_CORE, (core + 1) * B_CORE)
        m = {
            "tpk": _bf16(_pack_tp(inputs[sl], l_h[sl], r_h[sl])),
            "npk": _bf16(_pack_np(l_c[sl], r_c[sl])),
            "WxA": _bf16(WxA),
            "WhI": _bf16(WhI),
            "Whf": _bf16(W_fh),
        }
        if with_bias:
            m["biasA"] = _bf16(biasA)
            m["ones"] = _bf16(ones)
        in_maps.append(m)

    runner = _get_runner(with_bias)
    staged = runner.stage(in_maps)
    outs = runner.run(staged)
    per_core = runner.results(outs)
    LAST_RESULT = (runner, staged)
    hs, cs = [], []
    for d in per_core:
        h, c = _unpack_hc(d["hc"].astype(np.float32))
        hs.append(h)
        cs.append(c)
    return np.concatenate(hs, axis=0), np.concatenate(cs, axis=0)
